# revision 36
# baseline (speedup 1.0000x reference)
"""Cross-attention (LayerNorm -> MHA cross-attn -> out-proj -> residual) on 8 trn2 cores.

Sharding: core c -> (batch b = c//2, query-half qh = c%2). Each core computes all 16
heads for its 512 queries against the full 1024-token context of its batch. No
collectives needed; output shards are disjoint row blocks.

Host-side exact refactoring (all linear, fp32):
  - gamma folded into Wq:  Wq' = gamma[:,None] * Wq ;  qb = beta @ Wq
  - post-softmax scale 1/8 folded into Wv (power of two -> exact)
  - bo folded into the residual input: x' = x + bo
  - context shipped PRE-TRANSPOSED (ctxT = ctx.T, bf16)
  - wk/wv/wo shipped bf16 (psum accumulation stays fp32)

Device math per core:
  hn   = (x - mu) * rsqrt(var+eps)            (LayerNorm without affine)
  Q^T  = Wq'^T hn^T + qb
  K^T  = Wk^T ctx^T ;  V'' = [ctx Wv' | 1] with a ones column per head
  lT   = K^T_h (slice) x Q^T_h  per head      (logits, transposed layout [j, i])
  aT   = exp(lT)
  pv   = V''_h^T-contracted aT -> rows 0:64 = unnormalized vals, row 64 = Z
  vT   = valsU * (1/Z)  (recip on DVE per head-pair, junk rows harmless)
  y    = x' + vals @ Wo  (split: ct 0-3 accumulated during attention as PE filler)

v3: z-matmuls folded into pv via the ones column (-128 matmuls); out-proj split
P1/P2 so the tail stays PE-dense; weights bf16 so all weight DMAs issue at t=0.
"""

import numpy as np
import ml_dtypes
from contextlib import ExitStack

import concourse.bass as bass
import concourse.bacc as bacc_mod
import concourse.tile as tile
from concourse import mybir

F32 = mybir.dt.float32
F32R = mybir.dt.float32r
BF16 = mybir.dt.bfloat16
AF = mybir.ActivationFunctionType
ALU = mybir.AluOpType

B, NQ, NCTX, DQ, DC = 4, 1024, 1024, 1024, 768
H, DH, INNER = 16, 64, 1024
NQS = NQ // 2          # queries per core
EPS = 1e-5
N_CORES = 8


def _body(ctx, tc, nc, consts, bdiag, xp, ctxt_t, wq, qbr, wk, wv, wo, y):
    pers = ctx.enter_context(tc.tile_pool(name="pers", bufs=1))
    wpool = ctx.enter_context(tc.tile_pool(name="wpool", bufs=12))
    wqpool = ctx.enter_context(tc.tile_pool(name="wqpool", bufs=8))
    wopool = ctx.enter_context(tc.tile_pool(name="wopool", bufs=1))
    stat = ctx.enter_context(tc.tile_pool(name="stat", bufs=4))

    # ---- constants ----
    ident = pers.tile([128, 128], BF16, name="ident")
    eps_t = pers.tile([128, 1], F32, name="eps_t")
    nc.vector.memset(eps_t, EPS)
    zero_t = pers.tile([128, 1], F32, name="zero_t")
    nc.vector.memset(zero_t, 0.0)
    qbr_sb = pers.tile([1, INNER], BF16, name="qbr_sb")
    ones1 = pers.tile([1, 512], BF16, name="ones1")
    nc.vector.memset(ones1, 1.0)
    # ones-mask for the 1/Z partition broadcast (host-shipped, rows 1-63 zero):
    # row 0 has ones in cols 0:64 (head A), row 64 has ones in cols 64:128 (head B)
    onesBD = pers.tile([65, 128], F32, name="onesBD")

    # ---- persistent activations ----
    QT = pers.tile([128, 8, NQS], BF16, name="QT")          # Q^T  [inner, i]
    KT = pers.tile([128, 8, NCTX], BF16, name="KT")         # K^T  [inner, j]
    V_aug = pers.tile([128, 8, 16 * 65], BF16, name="V_aug")  # per head: [V_h | 1]
    nc.vector.memset(V_aug, 1.0)                            # ones cols stay 1.0
    valsT = pers.tile([128, 8, NQS], BF16, name="valsT")    # vals^T [inner, i]
    valsU = pers.tile([128, 8, NQS], BF16, name="valsU")    # unnormalized vals^T

    hT = pers.tile([128, 8, NQS], BF16, name="hT")          # hn^T  [d, i]
    e2 = tc.alloc_tile_pool(name="e2", bufs=1)
    # per-chunk tiles: dependency tracking is tile-granular, so one big tile
    # written by N DMAs would stall the first matmul on ALL N
    ctxT_t = [e2.tile([128, NCTX], BF16, name=f"ctxT{k}") for k in range(6)]
    xp_t = [pers.tile([128, 1, DQ], BF16, name=f"xp_sb{h}") for h in range(4)]
    xpr = xp.rearrange("(t p) d -> p t d", p=128)

    # ---- projection weights: chunked DMAs interleaved across the 3 DMA queues
    # (consecutive descriptors on a queue round-robin the 16 DMA engines; a
    # single big descriptor serializes on one engine)
    Q3 = [nc.sync, nc.scalar, nc.gpsimd]
    wv_t = [wpool.tile([128, INNER], BF16, tag="w", name=f"wv{k}") for k in range(6)]
    wk_t = [wpool.tile([128, INNER], BF16, tag="w", name=f"wk{k}") for k in range(6)]
    wq_t = [wqpool.tile([128, INNER], BF16, tag="wq", name=f"wq{k}") for k in range(8)]
    # One DMA descriptor runs on ONE engine (~20 B/ns): 256 KB = ~13 us. Split
    # the startup-critical ctxT/wv chunks into 4 x 64 KB pieces (parallel
    # engines, ~3 us each), round-robin the 3 issue queues, k-pairs in order.
    qi = 0
    for k in range(6):
        for p4 in range(4):
            j0, j1 = p4 * 256, (p4 + 1) * 256
            Q3[qi % 3].dma_start(out=ctxT_t[k][:, j0:j1],
                                 in_=ctxt_t[k * 128:(k + 1) * 128, j0:j1]); qi += 1
            Q3[qi % 3].dma_start(out=wv_t[k][:, j0:j1],
                                 in_=wv[k * 128:(k + 1) * 128, j0:j1]); qi += 1
    for it in range(4):  # xp: LN + transposes fill the projection DMA window
        Q3[qi % 3].dma_start(out=xp_t[it], in_=xpr[:, it:it + 1, :]); qi += 1
    for k in range(6):   # wk in halves (needed from ~30us)
        for p2 in range(2):
            e0, e1 = p2 * 512, (p2 + 1) * 512
            Q3[qi % 3].dma_start(out=wk_t[k][:, e0:e1],
                                 in_=wk[k * 128:(k + 1) * 128, e0:e1]); qi += 1
    for k in range(8):
        Q3[qi % 3].dma_start(out=wq_t[k], in_=wq[k * 128:(k + 1) * 128, :]); qi += 1
    # small constants at the queue tails (needed late)
    nc.sync.dma_start(out=qbr_sb, in_=qbr[0:1, :])
    nc.scalar.dma_start(out=ident, in_=consts[0:128, :])
    nc.gpsimd.dma_start(out=onesBD, in_=bdiag[0:65, :])

    # ---- V' = ctx @ Wv' -- k-major: 8 concurrent PSUM accumulation groups so
    # the first matmuls need only chunk 0 of ctxT+wv (PE starts with the DMA).
    # Own PSUM pool (all 8 banks), released before the attention tags claim them.
    pj = tc.alloc_tile_pool(name="pj", bufs=8, space="PSUM")
    for c in range(2):
        pvv = [pj.tile([128, 512], F32, tag="pj", name=f"pvv{c}_{jt}")
               for jt in range(8)]
        for k in range(6):
            for jt in range(8):
                nc.tensor.matmul(pvv[jt], ctxT_t[k][:, jt * 128:(jt + 1) * 128],
                                 wv_t[k][:, c * 512:(c + 1) * 512],
                                 start=(k == 0), stop=(k == 5))
        for jt in range(8):
            nc.vector.tensor_copy(
                out=V_aug[:, jt, c * 520:(c + 1) * 520]
                    .rearrange("p (h e) -> p h e", h=8)[:, :, 0:64],
                in_=pvv[jt].rearrange("p (h e) -> p h e", h=8),
            )

    # ---- LayerNorm (DVE; issued after V-proj so its FIFO slot does not
    # block the V-evacuations on late xp chunks) ----
    hn_t = []
    for it in range(4):
        st = stat.tile([128, 2, 6], F32, tag="st", name="st")
        for sb in range(2):
            nc.vector.bn_stats(out=st[:, sb, :],
                               in_=xp_t[it][:, 0, sb * 512:(sb + 1) * 512])
        mv = stat.tile([128, 2], F32, tag="mv", name="mv")
        nc.vector.bn_aggr(out=mv, in_=st)
        sd = stat.tile([128, 1], F32, tag="sd", name="sd")
        nc.scalar.activation(out=sd, in_=mv[:, 1:2], func=AF.Sqrt, bias=eps_t, scale=1.0)
        rstd = stat.tile([128, 1], F32, tag="rstd", name="rstd")
        nc.vector.reciprocal(out=rstd, in_=sd)
        nmu = stat.tile([128, 1], F32, tag="nmu", name="nmu")
        nc.vector.tensor_scalar(out=nmu, in0=mv[:, 0:1], scalar1=-1.0, scalar2=None, op0=ALU.mult)
        hn = stat.tile([128, DQ], BF16, tag="hn", bufs=4, name="hn")
        nc.vector.tensor_scalar(out=hn, in0=xp_t[it][:, 0, :], scalar1=nmu,
                                scalar2=rstd, op0=ALU.add, op1=ALU.mult)
        hn_t.append(hn)

    # ---- K^T = Wk^T @ ctx^T  (k-major as above) ----
    for c in range(2):
        pkk = [pj.tile([128, 512], F32, tag="pj", name=f"pkk{c}_{m}")
               for m in range(8)]
        for k in range(6):
            for m in range(8):
                nc.tensor.matmul(pkk[m], wk_t[k][:, m * 128:(m + 1) * 128],
                                 ctxT_t[k][:, c * 512:(c + 1) * 512],
                                 start=(k == 0), stop=(k == 5))
        for m in range(8):
            nc.scalar.activation(out=KT[:, m, c * 512:(c + 1) * 512], in_=pkk[m],
                                 func=AF.Copy, bias=0.0)
    pj.release()   # free all 8 PSUM banks for the attention-phase tags
    e2.release()
    ps = ctx.enter_context(tc.tile_pool(name="ps", bufs=2, space="PSUM"))
    # ---- transpose hn (PE, right after projections; hT copies on ACT) ----
    for it in range(4):
        for g in range(2):
            ptp = ps.tile([128, 4, 128], BF16, tag="mm", name="ptph")
            for q in range(4):
                dt_ = g * 4 + q
                nc.tensor.transpose(ptp[:, q, :], hn_t[it][:, dt_ * 128:(dt_ + 1) * 128], ident)
            nc.scalar.activation(
                out=hT[:, g * 4:(g + 1) * 4, it * 128:(it + 1) * 128],
                in_=ptp, func=AF.Copy, bias=0.0)

    att = tc.alloc_tile_pool(name="att", bufs=4)
    yac = None
    woR = []

    zg_t = [None] * 8

    def normalize(h):
        # broadcast 1/Z across partitions on the PE: one contract-65 fp32 matmul;
        # rows 0:64 <- zg[0] (maskA), 64:128 <- zg[64] (maskB); mask rows 1-63
        # are zero and zg rows 1-63 are finite (1.0-memset slots), contributing 0
        zps = ps.tile([128, 512], F32, tag="mm", name=f"zps{h}")
        nc.tensor.matmul(zps, onesBD, zg_t[h], start=True, stop=True)
        with nc.allow_low_precision(reason="softmax normalize in bf16, tol 2e-2"):
            nc.vector.tensor_mul(valsT[:, h, :], valsU[:, h, :], zps)

    # ---- attention: software-pipelined head pairs. ACT (exp, 8x965ns/pair)
    # is the pacer; pl/pq/pv matmuls are interleaved per j-tile so the PE
    # stream never blocks on a future dependency, and the next pair's QT is
    # evacuated mid-iteration so exp never waits at a pair boundary. ----
    for hp in range(8):
        hA, hB = 2 * hp, 2 * hp + 1
        aTP = att.tile([128, 8, 2, NQS], BF16, tag="aT", bufs=2, name=f"aTP{hp}")
        pq = ps.tile([128, 512], F32, tag="mm", name=f"pq{hp}")
        for k in range(8):
            nc.tensor.matmul(pq, wq_t[k][:, hp * 128:(hp + 1) * 128], hT[:, k, :],
                             start=(k == 0), stop=False)
        nc.tensor.matmul(pq, qbr_sb[0:1, hp * 128:(hp + 1) * 128], ones1,
                         start=False, stop=True)
        with nc.allow_low_precision(reason="QT bf16, tol 2e-2"):
            nc.vector.tensor_copy(out=QT[:, hp, :], in_=pq)
        pvA = ps.tile([65, 512], F32, tag="pva", bufs=1, name="pvA")
        pvB = ps.tile([65, 512], F32, tag="pvb", bufs=1, name="pvB")
        for jt in range(8):
            # logits pair: shares one PSUM tile; (0,0)/(64,0) row-tiled matmuls
            # issue adjacently and run concurrently in the PE array
            plP = ps.tile([128, 2, 512], F32, tag="lg", name="plP")
            nc.tensor.matmul(plP[:, 0, :], KT[0:64, hp, jt * 128:(jt + 1) * 128],
                             QT[0:64, hp, :], start=True, stop=True,
                             tile_position=(0, 0))
            nc.tensor.matmul(plP[:, 1, :], KT[64:128, hp, jt * 128:(jt + 1) * 128],
                             QT[64:128, hp, :], start=True, stop=True,
                             tile_position=(64, 0))
            nc.scalar.activation(out=aTP[:, jt, :, :], in_=plP, func=AF.Exp, bias=zero_t)
            if jt >= 1:
                jp = jt - 1
                nc.tensor.matmul(pvA, V_aug[:, jp, hA * 65:(hA + 1) * 65],
                                 aTP[:, jp, 0, :], start=(jp == 0), stop=False)
                nc.tensor.matmul(pvB, V_aug[:, jp, hB * 65:(hB + 1) * 65],
                                 aTP[:, jp, 1, :], start=(jp == 0), stop=False)
        for jp in (6, 7):
            nc.tensor.matmul(pvA, V_aug[:, jp, hA * 65:(hA + 1) * 65],
                             aTP[:, jp, 0, :], start=False, stop=(jp == 7))
            nc.tensor.matmul(pvB, V_aug[:, jp, hB * 65:(hB + 1) * 65],
                             aTP[:, jp, 1, :], start=False, stop=(jp == 7))
        if hp >= 2:
            # deferred by two pairs so the PE-stream position of zps(hp-2)
            # trails its DVE softmax chain by a whole iteration
            normalize(hp - 2)
        # softmax-denominator chain FIRST (it feeds the deferred zps and sits
        # ahead of the bulk valsU evacuation on the strict-FIFO DVE)
        zf = att.tile([65, 512], F32, tag="zf", bufs=3, name=f"zf{hp}")
        if hp < 3:
            # make rows 1-63 of each rotating buffer finite once; later writes
            # only touch rows 0 and 64, so recip stays finite on junk rows
            nc.vector.memset(zf, 1.0)
        nc.vector.tensor_copy(out=zf[0:1, :], in_=pvA[64:65, :])
        nc.vector.tensor_copy(out=zf[64:65, :], in_=pvB[64:65, :])
        zg = att.tile([65, 512], F32, tag="zg", bufs=3, name=f"zg{hp}")
        with nc.allow_low_precision(reason="softmax approx recip, tol 2e-2"):
            # recip must read SBUF (custom bitwise DVE op; PSUM read path breaks it)
            nc.vector.reciprocal_approx_fast(out=zg, in_=zf)
        zg_t[hp] = zg
        # evacuate: vals rows 0:64 per head; row 64 held the denominator
        nc.vector.tensor_copy(out=valsU[0:64, hp, :], in_=pvA[0:64, :])
        nc.vector.tensor_copy(out=valsU[64:128, hp, :], in_=pvB[0:64, :])

        if hp == 5:
            # out-proj weights, row-major: woR[ct] = wo[ct*128:(ct+1)*128, :]
            wo_sb = wopool.tile([128, 8, DQ], BF16, tag="wo", name="wo_sb")
            wor = wo.rearrange("(k p) e -> p k e", p=128)
            for ck in range(4):
                Q3[ck % 3].dma_start(out=wo_sb[:, 2 * ck:2 * ck + 2, :],
                                     in_=wor[:, 2 * ck:2 * ck + 2, :])
            for ct in range(8):
                woR.append(wo_sb[:, ct, :])
        if hp == 7:
            # P1: out-proj ct 0-5 (heads 0-11; normalized by the zps(5) issued
            # above) as PE filler; fold in the residual via a DVE add
            yac = att.tile([128, 2, 4, 512], BF16, tag="yac", bufs=1, name="yac")
            for c in range(2):
                for it in range(4):
                    po = ps.tile([128, 512], F32, tag="mm", name="po1")
                    for ct in range(6):
                        nc.tensor.matmul(po, valsT[:, ct, it * 128:(it + 1) * 128],
                                         woR[ct][:, c * 512:(c + 1) * 512],
                                         start=(ct == 0), stop=(ct == 5))
                    with nc.allow_low_precision(reason="partial out-proj sum bf16"):
                        nc.vector.tensor_add(yac[:, c, it, :], po,
                                             xp_t[it][:, 0, c * 512:(c + 1) * 512])

    # ---- P2: out-proj ct 6,7 + residual partials via identity matmul + store ----
    normalize(6)
    normalize(7)
    qengs = [nc.sync, nc.scalar, nc.gpsimd]
    for c in range(2):
        for it in range(4):
            po = ps.tile([128, 512], F32, tag="lg", name="po2")
            nc.tensor.matmul(po, valsT[:, 6, it * 128:(it + 1) * 128],
                             woR[6][:, c * 512:(c + 1) * 512], start=True, stop=False)
            nc.tensor.matmul(po, valsT[:, 7, it * 128:(it + 1) * 128],
                             woR[7][:, c * 512:(c + 1) * 512], start=False, stop=False)
            # residual+P1 partials folded in on the PE: po += I^T @ yac
            nc.tensor.matmul(po, ident, yac[:, c, it, :], start=False, stop=True)
            yt = att.tile([128, 512], F32, tag="yt", bufs=6, name="yt")
            if (c * 4 + it) % 2 == 0:
                nc.scalar.activation(out=yt, in_=po, func=AF.Copy, bias=0.0)
            else:
                nc.vector.tensor_copy(out=yt, in_=po)
            qengs[(c * 4 + it) % 3].dma_start(
                out=y.rearrange("(t p) d -> p t d", p=128)[:, it, c * 512:(c + 1) * 512],
                in_=yt)
    att.release()


def build_nc():
    nc = bacc_mod.Bacc()
    consts = nc.dram_tensor("consts", [128, 128], BF16, kind="ExternalInput")
    bdiag = nc.dram_tensor("bdiag", [65, 128], F32, kind="ExternalInput")
    xp = nc.dram_tensor("xp", [NQS, DQ], BF16, kind="ExternalInput")
    ctxt_t = nc.dram_tensor("ctxt_t", [DC, NCTX], BF16, kind="ExternalInput")
    wq = nc.dram_tensor("wq", [DQ, INNER], BF16, kind="ExternalInput")
    qbr = nc.dram_tensor("qbr", [1, INNER], BF16, kind="ExternalInput")
    wk = nc.dram_tensor("wk", [DC, INNER], BF16, kind="ExternalInput")
    wv = nc.dram_tensor("wv", [DC, INNER], BF16, kind="ExternalInput")
    wo = nc.dram_tensor("wo", [INNER, DQ], BF16, kind="ExternalInput")
    y = nc.dram_tensor("y", [NQS, DQ], F32, kind="ExternalOutput")
    with ExitStack() as ctx:
        tc = ctx.enter_context(tile.TileContext(nc))
        _body(ctx, tc, nc, consts, bdiag, xp, ctxt_t, wq, qbr, wk, wv, wo, y)
    nc.compile()
    return nc


def make_in_maps(x, context, Wq, Wk, Wv, Wo, bo, gamma, beta):
    x = np.asarray(x, np.float32)
    context = np.asarray(context, np.float32)
    Wq = np.asarray(Wq, np.float32)
    Wk = np.asarray(Wk, np.float32)
    Wv = np.asarray(Wv, np.float32)
    Wo = np.asarray(Wo, np.float32)
    bo = np.asarray(bo, np.float32)
    gamma = np.asarray(gamma, np.float32)
    beta = np.asarray(beta, np.float32)

    wq_f = np.ascontiguousarray((gamma[:, None] * Wq).astype(ml_dtypes.bfloat16))
    qb_f = np.ascontiguousarray((beta @ Wq)[None, :].astype(ml_dtypes.bfloat16))
    wv_bf = np.ascontiguousarray((Wv * np.float32(0.125)).astype(ml_dtypes.bfloat16))
    wk_bf = np.ascontiguousarray(Wk.astype(ml_dtypes.bfloat16))
    wo_bf = np.ascontiguousarray(Wo.astype(ml_dtypes.bfloat16))
    xp_full = (x + bo).astype(ml_dtypes.bfloat16)  # residual with bo folded in

    consts = np.eye(128).astype(ml_dtypes.bfloat16)
    bdiag = np.zeros((65, 128), np.float32)
    bdiag[0, 0:64] = 1.0       # broadcast row for partitions 0:64 (head A)
    bdiag[64, 64:128] = 1.0    # broadcast row for partitions 64:128 (head B)

    in_maps = []
    for c in range(N_CORES):
        b, qh = divmod(c, 2)
        in_maps.append({
            "consts": consts, "bdiag": bdiag,
            "xp": np.ascontiguousarray(xp_full[b, qh * NQS:(qh + 1) * NQS, :]),
            "ctxt_t": np.ascontiguousarray(context[b].T.astype(ml_dtypes.bfloat16)),
            "wq": wq_f, "qbr": qb_f, "wk": wk_bf, "wv": wv_bf,
            "wo": wo_bf,
        })
    return in_maps


_NC_CACHE = []


def kernel(x, context, Wq, Wk, Wv, Wo, bo, gamma, beta):
    from concourse.bass_utils import run_bass_kernel_spmd
    if not _NC_CACHE:
        _NC_CACHE.append(build_nc())
    nc = _NC_CACHE[0]
    in_maps = make_in_maps(x, context, Wq, Wk, Wv, Wo, bo, gamma, beta)
    res = run_bass_kernel_spmd(nc, in_maps, list(range(N_CORES)))
    y = np.empty((B, NQ, DQ), np.float32)
    for c in range(N_CORES):
        b, qh = divmod(c, 2)
        y[b, qh * NQS:(qh + 1) * NQS, :] = res.results[c]["y"]
    return y

